# revision 2
# baseline (speedup 1.0000x reference)
"""Banded multi-head attention (window +-64) for trn2, 8 NeuronCores.

Sharding: batch (2) x sequence blocks (4) -> 8 cores, no collectives.
Each core handles one 512-query block of one batch for ALL 16 heads:
  - loads x^T slices (with a 64-row K/V halo) and full projection weights
  - computes q^T/k^T (dk on partitions), V (seq on partitions)
  - banded scores per (head, 128-query chunk): [128q, 256t] tiles
  - softmax via exp (no max-sub needed: |s/8| < ~4), multiplicative band mask
  - P^T via PE transpose -> PV -> x_att^T -> output projection
Returns (out [2,2048,1024], attn [2,16,2048,2048]); attn is exactly zero
off-band, so the host scatters the compact [128,256] tiles into zeros.

Matmul operands are bf16 (host-cast); all accumulation is f32 in PSUM.
"""
import os
import numpy as np
import ml_dtypes
from contextlib import ExitStack

# NTFF tracing under axon needs antenv.axon_hooks, absent in slim agent
# containers; force-disable so a stray BASS_TRACE=1 can't crash the run.
os.environ["BASS_NEVER_TRACE"] = "1"

import concourse.bacc as bacc
import concourse.tile as tile
import concourse.mybir as mybir
from concourse import bass_utils

F32 = mybir.dt.float32
BF16 = mybir.dt.bfloat16
AF = mybir.ActivationFunctionType
ALU = mybir.AluOpType

B, S, D, H, DK = 2, 2048, 1024, 16, 64
NBLK = 4              # sequence blocks per batch (one per core pair-group)
SEQ = S // NBLK       # 512 queries per core
HALO = 64
KV = SEQ + 2 * HALO   # 640 key/value rows per core (zero-padded at edges)
NCH = SEQ // 128      # 4 query chunks of 128
G = D // 128          # 8 contraction chunks
NTB = KV // 128       # 5 V row chunks
SCALE = 1.0 / 8.0     # 1/sqrt(DK)

_NC = None
LAST_RESULTS = None


def _build():
    nc = bacc.Bacc("TRN2", target_bir_lowering=False, debug=False, num_devices=8)

    xqT_d = nc.dram_tensor("xqT", [D, SEQ], BF16, kind="ExternalInput").ap()
    xkT_d = nc.dram_tensor("xkT", [D, KV], BF16, kind="ExternalInput").ap()
    xvT_d = nc.dram_tensor("xvT", [D, KV], BF16, kind="ExternalInput").ap()
    wq_d = nc.dram_tensor("Wq", [D, D], BF16, kind="ExternalInput").ap()
    wk_d = nc.dram_tensor("Wk", [D, D], BF16, kind="ExternalInput").ap()
    wv_d = nc.dram_tensor("Wv", [D, D], BF16, kind="ExternalInput").ap()
    wo_d = nc.dram_tensor("Wo", [D, D], BF16, kind="ExternalInput").ap()
    bq_d = nc.dram_tensor("bq", [D], F32, kind="ExternalInput").ap()
    bk_d = nc.dram_tensor("bk", [D], F32, kind="ExternalInput").ap()
    bv_d = nc.dram_tensor("bv", [D], F32, kind="ExternalInput").ap()
    mask_d = nc.dram_tensor("mask", [NCH, 128, 256], F32, kind="ExternalInput").ap()
    id_d = nc.dram_tensor("ident", [128, 128], BF16, kind="ExternalInput").ap()

    attn_d = nc.dram_tensor("attn_c", [H, NCH, 128, 256], F32, kind="ExternalOutput").ap()
    out_d = nc.dram_tensor("out_d", [SEQ, D], F32, kind="ExternalOutput").ap()

    with tile.TileContext(nc) as tc, ExitStack() as ctx:
        persist = ctx.enter_context(tc.tile_pool(name="persist", bufs=1))
        work = ctx.enter_context(tc.tile_pool(name="work", bufs=3))
        ppool = ctx.enter_context(tc.tile_pool(name="pp", bufs=2, space="PSUM"))
        spool = ctx.enter_context(tc.tile_pool(name="ps", bufs=2, space="PSUM"))
        tpool = ctx.enter_context(tc.tile_pool(name="pt", bufs=2, space="PSUM"))
        vpool = ctx.enter_context(tc.tile_pool(name="pv", bufs=2, space="PSUM"))

        # persistent SBUF tensors; [128, G*N] = logical [D, N] in 128-row chunks
        xq_t = persist.tile([128, G * SEQ], BF16)
        xk_t = persist.tile([128, G * KV], BF16)
        xv_t = persist.tile([128, G * KV], BF16)
        wq_t = persist.tile([128, G * D], BF16)
        wk_t = persist.tile([128, G * D], BF16)
        wv_t = persist.tile([128, G * D], BF16)
        wo_t = persist.tile([128, G * D], BF16)
        qT = persist.tile([128, G * SEQ], BF16)   # [dk, q] 2 heads per chunk
        kT = persist.tile([128, G * KV], BF16)    # [dk, t]
        vS = persist.tile([128, NTB * D], BF16)   # [t, dv] per 128-row chunk
        xaT = persist.tile([128, G * SEQ], BF16)  # [dv, q]
        mk_t = persist.tile([128, NCH * 256], F32)
        id_t = persist.tile([128, 128], BF16)
        bq_t = persist.tile([128, G], F32)
        bk_t = persist.tile([128, G], F32)
        bv_t = persist.tile([128, G], F32)

        for g in range(G):
            nc.sync.dma_start(xq_t[:, g * SEQ:(g + 1) * SEQ], xqT_d[g * 128:(g + 1) * 128, :])
            nc.sync.dma_start(wq_t[:, g * D:(g + 1) * D], wq_d[g * 128:(g + 1) * 128, :])
        for g in range(G):
            nc.sync.dma_start(xk_t[:, g * KV:(g + 1) * KV], xkT_d[g * 128:(g + 1) * 128, :])
            nc.sync.dma_start(wk_t[:, g * D:(g + 1) * D], wk_d[g * 128:(g + 1) * 128, :])
        for g in range(G):
            nc.sync.dma_start(xv_t[:, g * KV:(g + 1) * KV], xvT_d[g * 128:(g + 1) * 128, :])
            nc.sync.dma_start(wv_t[:, g * D:(g + 1) * D], wv_d[g * 128:(g + 1) * 128, :])
        for g in range(G):
            nc.sync.dma_start(wo_t[:, g * D:(g + 1) * D], wo_d[g * 128:(g + 1) * 128, :])
        for c in range(NCH):
            nc.sync.dma_start(mk_t[:, c * 256:(c + 1) * 256], mask_d[c])
        nc.sync.dma_start(id_t[:], id_d)
        nc.sync.dma_start(bq_t[:], bq_d.rearrange("(m p) -> p m", p=128))
        nc.sync.dma_start(bk_t[:], bk_d.rearrange("(m p) -> p m", p=128))
        nc.sync.dma_start(bv_t[:], bv_d.rearrange("(m p) -> p m", p=128))

        # ---- Q projection: qT[m*128:(m+1)*128, :] = (Wq[:, m-slice].T @ xq) + bq
        for m in range(G):
            pq = ppool.tile([128, SEQ], F32, tag="pp")
            for g in range(G):
                nc.tensor.matmul(
                    pq[:],
                    wq_t[:, g * D + m * 128: g * D + (m + 1) * 128],
                    xq_t[:, g * SEQ:(g + 1) * SEQ],
                    start=(g == 0), stop=(g == G - 1),
                )
            nc.scalar.activation(qT[:, m * SEQ:(m + 1) * SEQ], pq[:],
                                 AF.Identity, bias=bq_t[:, m:m + 1], scale=1.0)

        # ---- K projection (two column passes: 512 + 128)
        for m in range(G):
            for c0, w in ((0, 512), (512, 128)):
                pk = ppool.tile([128, w], F32, tag="pp")
                for g in range(G):
                    nc.tensor.matmul(
                        pk[:],
                        wk_t[:, g * D + m * 128: g * D + (m + 1) * 128],
                        xk_t[:, g * KV + c0: g * KV + c0 + w],
                        start=(g == 0), stop=(g == G - 1),
                    )
                nc.scalar.activation(kT[:, m * KV + c0: m * KV + c0 + w], pk[:],
                                     AF.Identity, bias=bk_t[:, m:m + 1], scale=1.0)

        # ---- V projection: vS chunk tb = xv rows -> [t, dv] (bv folded in later)
        for tb in range(NTB):
            for n in range(2):
                pvp = ppool.tile([128, 512], F32, tag="pp")
                for g in range(G):
                    nc.tensor.matmul(
                        pvp[:],
                        xv_t[:, g * KV + tb * 128: g * KV + (tb + 1) * 128],
                        wv_t[:, g * D + n * 512: g * D + (n + 1) * 512],
                        start=(g == 0), stop=(g == G - 1),
                    )
                nc.vector.tensor_copy(vS[:, tb * D + n * 512: tb * D + (n + 1) * 512], pvp[:])

        # ---- attention + per-chunk output projection
        for c in range(NCH):
            for h in range(H):
                hp, m = (h % 2) * 64, h // 2
                ps_s = spool.tile([128, 256], F32, tag="ps")
                nc.tensor.matmul(
                    ps_s[:],
                    qT[hp:hp + 64, m * SEQ + c * 128: m * SEQ + (c + 1) * 128],
                    kT[hp:hp + 64, m * KV + c * 128: m * KV + c * 128 + 256],
                    start=True, stop=True,
                )
                pe = work.tile([128, 256], F32, tag="pexp")
                nc.scalar.activation(pe[:], ps_s[:], AF.Exp, scale=SCALE)
                pm = work.tile([128, 256], F32, tag="pm")
                den = work.tile([128, 1], F32, tag="den")
                nc.vector.scalar_tensor_tensor(
                    pm[:], pe[:], 1.0, mk_t[:, c * 256:(c + 1) * 256],
                    op0=ALU.mult, op1=ALU.mult, accum_out=den[:],
                )
                r = work.tile([128, 1], F32, tag="r")
                nc.vector.reciprocal(r[:], den[:])
                att_f = work.tile([128, 256], F32, tag="attf")
                nc.scalar.activation(att_f[:], pm[:], AF.Copy, scale=r[:])
                nc.sync.dma_start(attn_d[h, c], att_f[:])
                attb = work.tile([128, 256], BF16, tag="attb")
                nc.gpsimd.tensor_scalar_mul(attb[:], pm[:], r[:])

                ptbs = []
                for b2 in range(2):
                    ptp = tpool.tile([128, 128], BF16, tag="pt")
                    nc.tensor.transpose(ptp[:], attb[:, b2 * 128:(b2 + 1) * 128], id_t[:])
                    ptb = work.tile([128, 128], BF16, tag=f"ptb{b2}")
                    nc.vector.tensor_copy(ptb[:], ptp[:])
                    ptbs.append(ptb)
                xps = vpool.tile([64, 128], F32, tag="pv")
                for b2 in range(2):
                    nc.tensor.matmul(
                        xps[:],
                        vS[:, (c + b2) * D + h * 64: (c + b2) * D + (h + 1) * 64],
                        ptbs[b2][:],
                        start=(b2 == 0), stop=(b2 == 1),
                    )
                nc.scalar.activation(
                    xaT[hp:hp + 64, m * SEQ + c * 128: m * SEQ + (c + 1) * 128],
                    xps[:], AF.Identity, bias=bv_t[hp:hp + 64, m:m + 1], scale=1.0,
                )

            for n in range(2):
                po = ppool.tile([128, 512], F32, tag="pp")
                for g in range(G):
                    nc.tensor.matmul(
                        po[:],
                        xaT[:, g * SEQ + c * 128: g * SEQ + (c + 1) * 128],
                        wo_t[:, g * D + n * 512: g * D + (n + 1) * 512],
                        start=(g == 0), stop=(g == G - 1),
                    )
                ob = work.tile([128, 512], F32, tag="ob")
                nc.scalar.activation(ob[:], po[:], AF.Copy)
                nc.sync.dma_start(out_d[c * 128:(c + 1) * 128, n * 512:(n + 1) * 512], ob[:])

    nc.compile()
    return nc


def _shard_inputs(query, key, value, Wq, bq, Wk, bk, Wv, bv, Wo):
    bf = ml_dtypes.bfloat16
    wq = np.asarray(Wq, np.float32).astype(bf)
    wk = np.asarray(Wk, np.float32).astype(bf)
    wv = np.asarray(Wv, np.float32).astype(bf)
    wo = np.asarray(Wo, np.float32).astype(bf)
    bq32 = np.ascontiguousarray(np.asarray(bq, np.float32))
    bk32 = np.ascontiguousarray(np.asarray(bk, np.float32))
    bv32 = np.ascontiguousarray(np.asarray(bv, np.float32))
    ident = np.eye(128, dtype=bf)

    r_ = np.arange(128)[:, None]
    m_ = np.arange(256)[None, :]
    band = (m_ - r_ >= 0) & (m_ - r_ <= 2 * HALO)

    in_maps = []
    for core in range(8):
        b, blk = divmod(core, NBLK)
        q0 = blk * SEQ
        lo, hi = q0 - HALO, q0 + SEQ + HALO
        s0, s1 = max(lo, 0), min(hi, S)
        xq = np.asarray(query[b, q0:q0 + SEQ, :], np.float32).T.astype(bf)
        xk = np.zeros((D, KV), bf)
        xv = np.zeros((D, KV), bf)
        xk[:, s0 - lo:s1 - lo] = np.asarray(key[b, s0:s1, :], np.float32).T.astype(bf)
        xv[:, s0 - lo:s1 - lo] = np.asarray(value[b, s0:s1, :], np.float32).T.astype(bf)
        mask = np.zeros((NCH, 128, 256), np.float32)
        for c in range(NCH):
            j = lo + c * 128 + m_
            mask[c] = (band & (j >= 0) & (j < S)).astype(np.float32)
        in_maps.append({
            "xqT": xq, "xkT": xk, "xvT": xv,
            "Wq": wq, "Wk": wk, "Wv": wv, "Wo": wo,
            "bq": bq32, "bk": bk32, "bv": bv32,
            "mask": mask, "ident": ident,
        })
    return in_maps


def kernel(query, key, value, Wq, bq, Wk, bk, Wv, bv, Wo, bo):
    global _NC, LAST_RESULTS
    if _NC is None:
        _NC = _build()
    in_maps = _shard_inputs(query, key, value, Wq, bq, Wk, bk, Wv, bv, Wo)
    res = bass_utils.run_bass_kernel_spmd(_NC, in_maps, core_ids=list(range(8)))
    LAST_RESULTS = res

    bo32 = np.asarray(bo, np.float32)
    out = np.empty((B, S, D), np.float32)
    attn = np.zeros((B, H, S, S), np.float32)
    for core in range(8):
        rr = res.results[core]
        b, blk = divmod(core, NBLK)
        q0 = blk * SEQ
        out[b, q0:q0 + SEQ, :] = rr["out_d"] + bo32[None, :]
        ac = rr["attn_c"]  # [H, NCH, 128, 256]
        lo = q0 - HALO
        for c in range(NCH):
            j0 = lo + c * 128
            jlo, jhi = max(j0, 0), min(j0 + 256, S)
            attn[b, :, q0 + c * 128: q0 + (c + 1) * 128, jlo:jhi] = \
                ac[:, c, :, jlo - j0:jhi - j0]
    return out, attn
